# revision 20
# baseline (speedup 1.0000x reference)
"""Trainium2 Bass kernel for AnisotropicGNNLayer (kinematic-chain GNN layer).

Math (per batch b, frame f):
    diff[e]  = x[src[e]] - x[dst[e]]            src=[1..52], dst=[0..51]  (chain)
    msgs[e]  = diff[e] @ W[e]                   (E, Din, Dout) per-edge matmul
    agg[j]   = msgs[j] + pose[j]  (j<52);  agg[52] = pose[52]
    out      = gelu(LN(agg) * gamma + beta) + x @ res_W.T

Strategy: data-parallel over B across 8 NeuronCores (no collectives).

v2 design (vs. v1 baseline at ~685us/core measured):
 - Host pre-transposes f into [DIN, J*FRT] bf16 tiles: kills all on-chip PE
   transposes, PSUM->SBUF eviction copies and f32 input DMA (halves in-bytes).
 - Host centers W rows / pose so LN mean is exactly 0 (only sumsq needed).
 - Output DMA'd as bf16 (halves out-bytes), upcast to f32 on host.
 - Per 128-frame tile: one subtract (split DVE/GPSIMD) builds all edge diffs;
   flights of 12 joints over a shared 8-bank PSUM pool: edge matmul pairs +
   K=1 pose matmul, sumsq split across DVE (bn_stats/bn_aggr, ~2/3 of joints)
   and ACT (Square+accum), stats stored as mean/var PLANES (contiguous var
   columns keep the Newton APs fast), rstd via int bit-hack seed + 1 Newton
   iter on DVE (<=1.8e-3 rel err; eps dropped, negligible at var~2),
   per-joint ACT Gelu with scale=rstd straight from PSUM, residual matmul
   into pool banks, DVE add (gelu + residual) -> bf16, flight DMA out.
 - Root joint (no incoming edge): gelu(LN(pose)) is a host constant, added to
   the residual via K=1 matmul.

Fast path requires the chain graph; gamma!=1/beta!=0 handled by a slower
general path (extra per-joint DVE ops).  Non-chain graphs fall back to numpy.
"""

import sys

import numpy as np

if "/opt/trn_rl_repo" not in sys.path:
    sys.path.insert(0, "/opt/trn_rl_repo")

import ml_dtypes

B, FR, J, DIN, DOUT, E = 16, 512, 53, 128, 256, 52
EPS = 1e-5
NCORES = 8
FRAMES = B * FR                     # 8192
FPC = FRAMES // NCORES              # 1024 frames per core
FRT = 128                           # frames per tile (partition dim)
NT = FPC // FRT                     # 8 tiles per core
FLIGHT = 12                         # joints per flight (6 PSUM pair-banks)
MAGIC = 0x5F3759DF + 1              # fast-rsqrt seed (+1 folds the ~t negate)

_CACHE = {}


def _build(trivial_affine: bool, newton_engine="gpsimd", stats_dve_mod=3):
    """Build + compile the per-core Bass/Tile graph. SPMD: same graph, 8 cores.

    stats_dve_mod: joints with (j % stats_dve_mod != stats_dve_mod-1) get their
    sumsq on DVE (tensor_tensor_reduce), the rest on ACT (Square+accum).
    """
    import concourse.bacc as bacc
    import concourse.mybir as mybir
    import concourse.tile as tile
    from concourse.bass import ts

    f32 = mybir.dt.float32
    bf16 = mybir.dt.bfloat16
    i32 = mybir.dt.int32
    AF = mybir.ActivationFunctionType
    OP = mybir.AluOpType

    nc = bacc.Bacc("TRN2", target_bir_lowering=False, debug=False)

    f_d = nc.declare_dram_parameter("f", [NT * DIN, J * FRT], bf16, isOutput=False)
    w_d = nc.declare_dram_parameter("w", [DIN, E * DOUT], bf16, isOutput=False)
    rw_d = nc.declare_dram_parameter("rw", [DIN, DOUT], bf16, isOutput=False)
    pose_d = nc.declare_dram_parameter("pose", [1, J * DOUT], bf16, isOutput=False)
    g52_d = nc.declare_dram_parameter("g52", [1, DOUT], bf16, isOutput=False)
    if not trivial_affine:
        gam_d = nc.declare_dram_parameter("gam", [1, DOUT], f32, isOutput=False)
        bet_d = nc.declare_dram_parameter("bet", [1, DOUT], f32, isOutput=False)
    out_d = nc.declare_dram_parameter("out", [FPC, J * DOUT], bf16, isOutput=True)

    # flights of FLIGHT joints over the 52 edge-joints; root (52) rides in last
    flights = [(g0, min(g0 + FLIGHT, E)) for g0 in range(0, E, FLIGHT)]

    nwt = getattr(nc, {"gpsimd": "gpsimd", "vector": "vector"}[newton_engine])

    with tile.TileContext(nc) as tc:
        with (
            tc.tile_pool(name="singles", bufs=1) as singles,
            tc.tile_pool(name="fpool", bufs=3) as fpool,
            tc.tile_pool(name="dpool", bufs=3) as dpool,
            tc.tile_pool(name="statpool", bufs=2) as statpool,
            tc.tile_pool(name="nwts", bufs=2) as nwts,
            tc.tile_pool(name="scrap", bufs=6) as scrap,
            tc.tile_pool(name="gpool", bufs=4) as gpool,
            tc.tile_pool(name="opool", bufs=3) as opool,
            tc.tile_pool(name="ps", bufs=8, space="PSUM") as ps,
        ):
            w_sb = singles.tile([DIN, E * DOUT], bf16)
            nc.sync.dma_start(out=w_sb, in_=w_d[:, :])
            rw_sb = singles.tile([DIN, DOUT], bf16)
            nc.sync.dma_start(out=rw_sb, in_=rw_d[:, :])
            pose_sb = singles.tile([1, J * DOUT], bf16)
            nc.sync.dma_start(out=pose_sb, in_=pose_d[:, :])
            ones_sb = singles.tile([1, DIN], bf16)
            nc.vector.memset(ones_sb, 1.0)
            g52_sb = singles.tile([1, DOUT], bf16)
            nc.sync.dma_start(out=g52_sb, in_=g52_d[:, :])
            if not trivial_affine:
                import concourse.bass as bass

                gam_sb = singles.tile([FRT, DOUT], f32)
                nc.gpsimd.dma_start(
                    out=gam_sb,
                    in_=bass.AP(
                        tensor=gam_d.tensor,
                        offset=gam_d.offset,
                        ap=[[0, FRT], gam_d.ap[1]],
                    ),
                )
                bet_sb = singles.tile([FRT, DOUT], f32)
                nc.gpsimd.dma_start(
                    out=bet_sb,
                    in_=bass.AP(
                        tensor=bet_d.tensor,
                        offset=bet_d.offset,
                        ap=[[0, FRT], bet_d.ap[1]],
                    ),
                )

            for t in range(NT):
                r0 = t * FRT
                fT = fpool.tile([DIN, J * FRT], bf16, tag="fT")
                nc.sync.dma_start(out=fT, in_=f_d[t * DIN : (t + 1) * DIN, :])
                diffT = dpool.tile([DIN, E * FRT], bf16, tag="diffT")
                half = (E * FRT) // 2
                nc.vector.tensor_tensor(
                    out=diffT[:, :half],
                    in0=fT[:, FRT : FRT + half],
                    in1=fT[:, :half],
                    op=OP.subtract,
                )
                nc.gpsimd.tensor_tensor(
                    out=diffT[:, half:],
                    in0=fT[:, FRT + half :],
                    in1=fT[:, half : E * FRT],
                    op=OP.subtract,
                )

                # mv: per joint j, col 2j = mean (bn; unused), 2j+1 = var
                mv = statpool.tile([FRT, 128], f32, tag="mv")
                rstd = statpool.tile([FRT, 64], f32, tag="rstd")

                for g0, g1 in flights:
                    gn = g1 - g0
                    is_last = g1 == E
                    # ---- edge matmuls + pose into pair banks ----
                    pxs = {}
                    for j0 in range(g0, g1, 2):
                        pn = min(2, g1 - j0)
                        px = ps.tile([FRT, 512], f32, tag="ps")
                        pxs[j0] = px
                        for k in range(pn):
                            j = j0 + k
                            # start only on the first matmul of the bank:
                            # start=True clears has_written for the WHOLE bank
                            nc.tensor.matmul(
                                px[:, k * DOUT : (k + 1) * DOUT],
                                lhsT=diffT[:, ts(j, FRT)],
                                rhs=w_sb[:, ts(j, DOUT)],
                                start=(k == 0),
                                stop=False,
                            )
                        nc.tensor.matmul(
                            px[:, : pn * DOUT],
                            lhsT=ones_sb,
                            rhs=pose_sb[:, j0 * DOUT : (j0 + pn) * DOUT],
                            start=False,
                            stop=True,
                        )
                        # ---- sumsq -> mv (var); mean is exactly 0 ----
                        for k in range(pn):
                            j = j0 + k
                            sl = slice(k * DOUT, (k + 1) * DOUT)
                            if (j % 9) < 5:
                                st6 = scrap.tile([FRT, 6], f32, tag="st6")
                                nc.vector.bn_stats(out=st6, in_=px[:, sl])
                                # out: mean -> col j, var -> col 64+j (planes)
                                nc.vector.bn_aggr(
                                    out=mv.rearrange(
                                        "p (two g) -> p two g", two=2
                                    )[:, :, j : j + 1],
                                    in_=st6,
                                )
                            else:
                                dmp = scrap.tile([FRT, DOUT], bf16, tag="dmpA")
                                nc.scalar.activation(
                                    out=dmp,
                                    in_=px[:, sl],
                                    func=AF.Square,
                                    scale=1.0 / 16.0,
                                    accum_out=mv[:, 64 + j : 64 + j + 1],
                                )

                    # ---- rstd for the flight: bit-hack seed + 1 Newton ----
                    # (eps dropped: |eps/var| ~ 5e-6 for this data regime;
                    #  1 NR iter leaves <=1.8e-3 rel err on rstd)
                    gsl = slice(g0, g1)
                    var_g = mv[:, 64 + g0 : 64 + g1]
                    na = nwts.tile([FRT, FLIGHT], f32, tag="na")
                    ti = nwts.tile([FRT, FLIGHT], i32, tag="ti")
                    nc.vector.tensor_scalar(
                        out=ti[:, :gn], in0=var_g.bitcast(i32),
                        scalar1=1, scalar2=-1,
                        op0=OP.logical_shift_right, op1=OP.bitwise_xor,
                    )
                    nc.vector.tensor_scalar(
                        out=rstd[:, gsl].bitcast(i32), in0=ti[:, :gn],
                        scalar1=MAGIC, scalar2=0, op0=OP.add, op1=OP.add,
                    )
                    for _ in range(1):
                        nc.vector.tensor_tensor(
                            out=na[:, :gn], in0=rstd[:, gsl], in1=rstd[:, gsl],
                            op=OP.mult,
                        )
                        nc.vector.scalar_tensor_tensor(
                            out=na[:, :gn], in0=na[:, :gn], scalar=-0.5,
                            in1=var_g, op0=OP.mult, op1=OP.mult,
                        )
                        nc.vector.scalar_tensor_tensor(
                            out=rstd[:, gsl], in0=na[:, :gn], scalar=1.5,
                            in1=rstd[:, gsl], op0=OP.add, op1=OP.mult,
                        )

                    # ---- gelu + residual + add ----
                    on = gn + (1 if is_last else 0)   # +root in last flight
                    outS = opool.tile([FRT, FLIGHT * DOUT], bf16, tag="outS")
                    for j0 in range(g0, g1, 2):
                        pn = min(2, g1 - j0)
                        px = pxs[j0]
                        gS = gpool.tile([FRT, 512], bf16, tag="gS")
                        pr = ps.tile([FRT, 512], f32, tag="ps")
                        for k in range(pn):
                            j = j0 + k
                            sl = slice(k * DOUT, (k + 1) * DOUT)
                            if trivial_affine:
                                nc.scalar.activation(
                                    out=gS[:, sl],
                                    in_=px[:, sl],
                                    func=AF.Gelu,
                                    scale=rstd[:, j : j + 1],
                                )
                            else:
                                xh = scrap.tile([FRT, DOUT], f32, tag="xhat")
                                nc.scalar.activation(
                                    out=xh, in_=px[:, sl], func=AF.Copy,
                                    scale=rstd[:, j : j + 1],
                                )
                                nc.vector.tensor_tensor(
                                    out=xh, in0=xh, in1=gam_sb, op=OP.mult
                                )
                                nc.vector.tensor_tensor(
                                    out=xh, in0=xh, in1=bet_sb, op=OP.add
                                )
                                nc.scalar.activation(out=gS[:, sl], in_=xh, func=AF.Gelu)
                            nc.tensor.matmul(
                                pr[:, sl],
                                lhsT=fT[:, ts(j, FRT)],
                                rhs=rw_sb,
                                start=(k == 0),
                                stop=(k == pn - 1),
                            )
                        psl = slice(0, pn * DOUT)
                        osl = slice((j0 - g0) * DOUT, (j0 - g0 + pn) * DOUT)
                        nc.vector.tensor_tensor(
                            out=outS[:, osl], in0=gS[:, psl], in1=pr[:, psl], op=OP.add
                        )
                    if is_last:
                        # root joint: residual + host-constant gelu(LN(pose))
                        pr = ps.tile([FRT, 512], f32, tag="ps")
                        nc.tensor.matmul(
                            pr[:, :DOUT],
                            lhsT=fT[:, ts(J - 1, FRT)],
                            rhs=rw_sb,
                            start=True,
                            stop=False,
                        )
                        nc.tensor.matmul(
                            pr[:, :DOUT],
                            lhsT=ones_sb,
                            rhs=g52_sb[:, :],
                            start=False,
                            stop=True,
                        )
                        nc.scalar.copy(
                            out=outS[:, gn * DOUT : on * DOUT], in_=pr[:, :DOUT]
                        )
                    nc.sync.dma_start(
                        out=out_d[r0 : r0 + FRT, g0 * DOUT : (g0 + on) * DOUT],
                        in_=outS[:, : on * DOUT],
                    )

    nc.compile()
    return nc


def _get_nc(trivial_affine: bool):
    key = ("nc", trivial_affine)
    if key not in _CACHE:
        _CACHE[key] = _build(trivial_affine)
    return _CACHE[key]


def _numpy_fallback(f, W, pose_emb, gamma, beta, res_W, src, dst):
    f64 = f.astype(np.float32)
    diff = f64[:, :, src, :] - f64[:, :, dst, :]
    msgs = np.einsum("bfei,eio->bfeo", diff, W)
    agg = np.zeros(f.shape[:3] + (W.shape[-1],), np.float32)
    np.add.at(agg, (slice(None), slice(None), dst), msgs)
    agg = agg + pose_emb
    mu = agg.mean(-1, keepdims=True)
    var = ((agg - mu) ** 2).mean(-1, keepdims=True)
    normed = (agg - mu) / np.sqrt(var + EPS) * gamma + beta
    res = np.einsum("bfji,oi->bfjo", f64, res_W)
    from scipy.special import erf  # noqa: PLC0415

    gelu = normed * 0.5 * (1.0 + erf(normed / np.sqrt(2.0)))
    return (gelu + res).astype(np.float32)


def kernel(f, W, pose_emb, gamma, beta, res_W, src, dst):
    f = np.asarray(f)
    W = np.asarray(W, np.float32)
    pose_emb = np.asarray(pose_emb, np.float32)
    gamma = np.asarray(gamma, np.float32)
    beta = np.asarray(beta, np.float32)
    res_W = np.asarray(res_W, np.float32)
    src = np.asarray(src)
    dst = np.asarray(dst)

    chain = np.array_equal(src, np.arange(1, J)) and np.array_equal(
        dst, np.arange(0, J - 1)
    )
    if not chain or f.shape != (B, FR, J, DIN):
        return _numpy_fallback(f, W, pose_emb, gamma, beta, res_W, src, dst)

    trivial_affine = bool(
        np.all(gamma == gamma.flat[0])
        and abs(gamma.flat[0] - 1.0) < 1e-12
        and np.all(beta == 0.0)
    )

    # Host prep: center W rows / pose so on-chip LN mean is exactly 0.
    Wc = W - W.mean(axis=2, keepdims=True)              # (E, Din, Dout)
    pc = pose_emb - pose_emb.mean(axis=1, keepdims=True)  # (J, Dout)
    w_host = np.ascontiguousarray(Wc.transpose(1, 0, 2).reshape(DIN, E * DOUT)).astype(
        ml_dtypes.bfloat16
    )
    # root joint (no incoming edge): gelu(LN(pose_52)*gamma+beta) is constant
    p52 = pc[J - 1].astype(np.float64)
    n52 = p52 / np.sqrt((p52 ** 2).mean() + EPS) * gamma.astype(np.float64) + beta
    from scipy.special import erf  # noqa: PLC0415

    g52 = (n52 * 0.5 * (1.0 + erf(n52 / np.sqrt(2.0)))).astype(np.float32)
    g52_host = g52.reshape(1, DOUT).astype(ml_dtypes.bfloat16)
    rw_host = np.ascontiguousarray(res_W.T).astype(ml_dtypes.bfloat16)  # (Din, Dout)
    pose_host = pc.reshape(1, J * DOUT).astype(ml_dtypes.bfloat16)

    # pre-transpose f: [core][tile*DIN, J*FRT] bf16 (DIN on partitions)
    f_bf = f.reshape(NCORES, NT, FRT, J, DIN).astype(ml_dtypes.bfloat16)
    f_host = np.ascontiguousarray(f_bf.transpose(0, 1, 4, 3, 2)).reshape(
        NCORES, NT * DIN, J * FRT
    )

    from concourse.bass_utils import run_bass_kernel_spmd  # noqa: PLC0415

    nc = _get_nc(trivial_affine)
    in_maps = []
    for c in range(NCORES):
        m = {
            "f": f_host[c],
            "w": w_host,
            "rw": rw_host,
            "pose": pose_host,
            "g52": g52_host,
        }
        if not trivial_affine:
            m["gam"] = gamma.reshape(1, DOUT)
            m["bet"] = beta.reshape(1, DOUT)
        in_maps.append(m)

    _CACHE["last_in_maps"] = in_maps
    res = run_bass_kernel_spmd(nc, in_maps, core_ids=list(range(NCORES)))
    outs = [
        res.results[c]["out"].astype(np.float32).reshape(FPC, J, DOUT)
        for c in range(NCORES)
    ]
    return np.concatenate(outs, axis=0).reshape(B, FR, J, DOUT)


# revision 21
# speedup vs baseline: 1.2220x; 1.2220x over previous
"""Trainium2 Bass kernel for AnisotropicGNNLayer (kinematic-chain GNN layer).

Math (per batch b, frame f):
    diff[e]  = x[src[e]] - x[dst[e]]            src=[1..52], dst=[0..51]  (chain)
    msgs[e]  = diff[e] @ W[e]                   (E, Din, Dout) per-edge matmul
    agg[j]   = msgs[j] + pose[j]  (j<52);  agg[52] = pose[52]
    out      = gelu(LN(agg) * gamma + beta) + x @ res_W.T

Strategy: data-parallel over B across 8 NeuronCores (no collectives).

v2 design (vs. v1 baseline at ~685us/core measured):
 - Host pre-transposes f into [DIN, J*FRT] bf16 tiles: kills all on-chip PE
   transposes, PSUM->SBUF eviction copies and f32 input DMA (halves in-bytes).
 - Host centers W rows / pose so LN mean is exactly 0 (only sumsq needed).
 - Output DMA'd as bf16 (halves out-bytes), upcast to f32 on host.
 - Per 128-frame tile: one subtract (split DVE/GPSIMD) builds all edge diffs;
   flights of 12 joints over a shared 8-bank PSUM pool: edge matmul pairs +
   K=1 pose matmul, sumsq split across DVE (bn_stats/bn_aggr, ~2/3 of joints)
   and ACT (Square+accum), stats stored as mean/var PLANES (contiguous var
   columns keep the Newton APs fast), rstd via int bit-hack seed + 1 Newton
   iter on DVE (<=1.8e-3 rel err; eps dropped, negligible at var~2),
   per-joint ACT Gelu with scale=rstd straight from PSUM, residual matmul
   into pool banks, DVE add (gelu + residual) -> bf16, flight DMA out.
 - Root joint (no incoming edge): gelu(LN(pose)) is a host constant, added to
   the residual via K=1 matmul.

Fast path requires the chain graph; gamma!=1/beta!=0 handled by a slower
general path (extra per-joint DVE ops).  Non-chain graphs fall back to numpy.
"""

import sys

import numpy as np

if "/opt/trn_rl_repo" not in sys.path:
    sys.path.insert(0, "/opt/trn_rl_repo")

import ml_dtypes

B, FR, J, DIN, DOUT, E = 16, 512, 53, 128, 256, 52
EPS = 1e-5
NCORES = 8
FRAMES = B * FR                     # 8192
FPC = FRAMES // NCORES              # 1024 frames per core
FRT = 128                           # frames per tile (partition dim)
NT = FPC // FRT                     # 8 tiles per core
FLIGHT = 12                         # joints per flight (6 PSUM pair-banks)
MAGIC = 0x5F3759DF + 1              # fast-rsqrt seed (+1 folds the ~t negate)

_CACHE = {}


def _build(trivial_affine: bool, newton_engine="gpsimd", stats_dve_mod=3):
    """Build + compile the per-core Bass/Tile graph. SPMD: same graph, 8 cores.

    stats_dve_mod: joints with (j % stats_dve_mod != stats_dve_mod-1) get their
    sumsq on DVE (tensor_tensor_reduce), the rest on ACT (Square+accum).
    """
    import concourse.bacc as bacc
    import concourse.mybir as mybir
    import concourse.tile as tile
    from concourse.bass import ts

    f32 = mybir.dt.float32
    bf16 = mybir.dt.bfloat16
    i32 = mybir.dt.int32
    AF = mybir.ActivationFunctionType
    OP = mybir.AluOpType

    nc = bacc.Bacc("TRN2", target_bir_lowering=False, debug=False)

    f_d = nc.declare_dram_parameter("f", [NT * DIN, J * FRT], bf16, isOutput=False)
    w_d = nc.declare_dram_parameter("w", [DIN, E * DOUT], bf16, isOutput=False)
    rw_d = nc.declare_dram_parameter("rw", [DIN, DOUT], bf16, isOutput=False)
    pose_d = nc.declare_dram_parameter("pose", [1, J * DOUT], bf16, isOutput=False)
    g52_d = nc.declare_dram_parameter("g52", [1, DOUT], bf16, isOutput=False)
    if not trivial_affine:
        gam_d = nc.declare_dram_parameter("gam", [1, DOUT], f32, isOutput=False)
        bet_d = nc.declare_dram_parameter("bet", [1, DOUT], f32, isOutput=False)
    out_d = nc.declare_dram_parameter("out", [FPC, J * DOUT], bf16, isOutput=True)

    # flights of FLIGHT joints over the 52 edge-joints; root (52) rides in last
    flights = [(g0, min(g0 + FLIGHT, E)) for g0 in range(0, E, FLIGHT)]

    nwt = getattr(nc, {"gpsimd": "gpsimd", "vector": "vector"}[newton_engine])

    with tile.TileContext(nc) as tc:
        with (
            tc.tile_pool(name="singles", bufs=1) as singles,
            tc.tile_pool(name="fpool", bufs=2) as fpool,
            tc.tile_pool(name="dpool", bufs=2) as dpool,
            tc.tile_pool(name="statpool", bufs=2) as statpool,
            tc.tile_pool(name="nwts", bufs=2) as nwts,
            tc.tile_pool(name="scrap", bufs=6) as scrap,
            tc.tile_pool(name="gpool", bufs=4) as gpool,
            tc.tile_pool(name="opool", bufs=3) as opool,
            tc.tile_pool(name="ps", bufs=8, space="PSUM") as ps,
        ):
            w_sb = singles.tile([DIN, E * DOUT], bf16)
            nc.sync.dma_start(out=w_sb, in_=w_d[:, :])
            rw_sb = singles.tile([DIN, DOUT], bf16)
            nc.sync.dma_start(out=rw_sb, in_=rw_d[:, :])
            pose_sb = singles.tile([1, J * DOUT], bf16)
            nc.sync.dma_start(out=pose_sb, in_=pose_d[:, :])
            ones_sb = singles.tile([1, DIN], bf16)
            nc.vector.memset(ones_sb, 1.0)
            g52_sb = singles.tile([1, DOUT], bf16)
            nc.sync.dma_start(out=g52_sb, in_=g52_d[:, :])
            if not trivial_affine:
                import concourse.bass as bass

                gam_sb = singles.tile([FRT, DOUT], f32)
                nc.gpsimd.dma_start(
                    out=gam_sb,
                    in_=bass.AP(
                        tensor=gam_d.tensor,
                        offset=gam_d.offset,
                        ap=[[0, FRT], gam_d.ap[1]],
                    ),
                )
                bet_sb = singles.tile([FRT, DOUT], f32)
                nc.gpsimd.dma_start(
                    out=bet_sb,
                    in_=bass.AP(
                        tensor=bet_d.tensor,
                        offset=bet_d.offset,
                        ap=[[0, FRT], bet_d.ap[1]],
                    ),
                )

            for t in range(NT):
                r0 = t * FRT
                fT = fpool.tile([DIN, J * FRT], bf16, tag="fT")
                nc.sync.dma_start(out=fT, in_=f_d[t * DIN : (t + 1) * DIN, :])
                diffT = dpool.tile([DIN, E * FRT], bf16, tag="diffT")
                half = (E * FRT) // 2
                nc.vector.tensor_tensor(
                    out=diffT[:, :half],
                    in0=fT[:, FRT : FRT + half],
                    in1=fT[:, :half],
                    op=OP.subtract,
                )
                nc.gpsimd.tensor_tensor(
                    out=diffT[:, half:],
                    in0=fT[:, FRT + half :],
                    in1=fT[:, half : E * FRT],
                    op=OP.subtract,
                )

                # mv: per joint j, col 2j = mean (bn; unused), 2j+1 = var
                mv = statpool.tile([FRT, 128], f32, tag="mv")
                rstd = statpool.tile([FRT, 64], f32, tag="rstd")

                for g0, g1 in flights:
                    gn = g1 - g0
                    is_last = g1 == E
                    # ---- edge matmuls + pose into pair banks ----
                    pxs = {}
                    for j0 in range(g0, g1, 2):
                        pn = min(2, g1 - j0)
                        px = ps.tile([FRT, 512], f32, tag="ps")
                        pxs[j0] = px
                        for k in range(pn):
                            j = j0 + k
                            # start only on the first matmul of the bank:
                            # start=True clears has_written for the WHOLE bank
                            nc.tensor.matmul(
                                px[:, k * DOUT : (k + 1) * DOUT],
                                lhsT=diffT[:, ts(j, FRT)],
                                rhs=w_sb[:, ts(j, DOUT)],
                                start=(k == 0),
                                stop=False,
                            )
                        nc.tensor.matmul(
                            px[:, : pn * DOUT],
                            lhsT=ones_sb,
                            rhs=pose_sb[:, j0 * DOUT : (j0 + pn) * DOUT],
                            start=False,
                            stop=True,
                        )
                        # ---- sumsq -> mv (var); mean is exactly 0 ----
                        for k in range(pn):
                            j = j0 + k
                            sl = slice(k * DOUT, (k + 1) * DOUT)
                            if (j % 3) != 2:
                                st6 = scrap.tile([FRT, 6], f32, tag="st6")
                                nc.vector.bn_stats(out=st6, in_=px[:, sl])
                                # out: mean -> col j, var -> col 64+j (planes)
                                nc.vector.bn_aggr(
                                    out=mv.rearrange(
                                        "p (two g) -> p two g", two=2
                                    )[:, :, j : j + 1],
                                    in_=st6,
                                )
                            else:
                                dmp = scrap.tile([FRT, DOUT], bf16, tag="dmpA")
                                nc.scalar.activation(
                                    out=dmp,
                                    in_=px[:, sl],
                                    func=AF.Square,
                                    scale=1.0 / 16.0,
                                    accum_out=mv[:, 64 + j : 64 + j + 1],
                                )

                    # ---- rstd for the flight: bit-hack seed + 1 Newton ----
                    # (eps dropped: |eps/var| ~ 5e-6 for this data regime;
                    #  1 NR iter leaves <=1.8e-3 rel err on rstd)
                    gsl = slice(g0, g1)
                    var_g = mv[:, 64 + g0 : 64 + g1]
                    na = nwts.tile([FRT, FLIGHT], f32, tag="na")
                    ti = nwts.tile([FRT, FLIGHT], i32, tag="ti")
                    nc.vector.tensor_scalar(
                        out=ti[:, :gn], in0=var_g.bitcast(i32),
                        scalar1=1, scalar2=-1,
                        op0=OP.logical_shift_right, op1=OP.bitwise_xor,
                    )
                    nc.vector.tensor_scalar(
                        out=rstd[:, gsl].bitcast(i32), in0=ti[:, :gn],
                        scalar1=MAGIC, scalar2=0, op0=OP.add, op1=OP.add,
                    )
                    for _ in range(1):
                        nc.vector.tensor_tensor(
                            out=na[:, :gn], in0=rstd[:, gsl], in1=rstd[:, gsl],
                            op=OP.mult,
                        )
                        nc.vector.scalar_tensor_tensor(
                            out=na[:, :gn], in0=na[:, :gn], scalar=-0.5,
                            in1=var_g, op0=OP.mult, op1=OP.mult,
                        )
                        nc.vector.scalar_tensor_tensor(
                            out=rstd[:, gsl], in0=na[:, :gn], scalar=1.5,
                            in1=rstd[:, gsl], op0=OP.add, op1=OP.mult,
                        )

                    # ---- gelu + residual + add ----
                    on = gn + (1 if is_last else 0)   # +root in last flight
                    outS = opool.tile([FRT, FLIGHT * DOUT], bf16, tag="outS")
                    for j0 in range(g0, g1, 2):
                        pn = min(2, g1 - j0)
                        px = pxs[j0]
                        gS = gpool.tile([FRT, 512], bf16, tag="gS")
                        pr = ps.tile([FRT, 512], f32, tag="ps")
                        for k in range(pn):
                            j = j0 + k
                            sl = slice(k * DOUT, (k + 1) * DOUT)
                            if trivial_affine:
                                nc.scalar.activation(
                                    out=gS[:, sl],
                                    in_=px[:, sl],
                                    func=AF.Gelu,
                                    scale=rstd[:, j : j + 1],
                                )
                            else:
                                xh = scrap.tile([FRT, DOUT], f32, tag="xhat")
                                nc.scalar.activation(
                                    out=xh, in_=px[:, sl], func=AF.Copy,
                                    scale=rstd[:, j : j + 1],
                                )
                                nc.vector.tensor_tensor(
                                    out=xh, in0=xh, in1=gam_sb, op=OP.mult
                                )
                                nc.vector.tensor_tensor(
                                    out=xh, in0=xh, in1=bet_sb, op=OP.add
                                )
                                nc.scalar.activation(out=gS[:, sl], in_=xh, func=AF.Gelu)
                            nc.tensor.matmul(
                                pr[:, sl],
                                lhsT=fT[:, ts(j, FRT)],
                                rhs=rw_sb,
                                start=(k == 0),
                                stop=(k == pn - 1),
                            )
                        psl = slice(0, pn * DOUT)
                        osl = slice((j0 - g0) * DOUT, (j0 - g0 + pn) * DOUT)
                        nc.vector.tensor_tensor(
                            out=outS[:, osl], in0=gS[:, psl], in1=pr[:, psl], op=OP.add
                        )
                    if is_last:
                        # root joint: residual + host-constant gelu(LN(pose))
                        pr = ps.tile([FRT, 512], f32, tag="ps")
                        nc.tensor.matmul(
                            pr[:, :DOUT],
                            lhsT=fT[:, ts(J - 1, FRT)],
                            rhs=rw_sb,
                            start=True,
                            stop=False,
                        )
                        nc.tensor.matmul(
                            pr[:, :DOUT],
                            lhsT=ones_sb,
                            rhs=g52_sb[:, :],
                            start=False,
                            stop=True,
                        )
                        nc.vector.tensor_copy(
                            outS[:, gn * DOUT : on * DOUT], pr[:, :DOUT]
                        )
                    nc.sync.dma_start(
                        out=out_d[r0 : r0 + FRT, g0 * DOUT : (g0 + on) * DOUT],
                        in_=outS[:, : on * DOUT],
                    )

    nc.compile()
    return nc


def _get_nc(trivial_affine: bool):
    key = ("nc", trivial_affine)
    if key not in _CACHE:
        _CACHE[key] = _build(trivial_affine)
    return _CACHE[key]


def _numpy_fallback(f, W, pose_emb, gamma, beta, res_W, src, dst):
    f64 = f.astype(np.float32)
    diff = f64[:, :, src, :] - f64[:, :, dst, :]
    msgs = np.einsum("bfei,eio->bfeo", diff, W)
    agg = np.zeros(f.shape[:3] + (W.shape[-1],), np.float32)
    np.add.at(agg, (slice(None), slice(None), dst), msgs)
    agg = agg + pose_emb
    mu = agg.mean(-1, keepdims=True)
    var = ((agg - mu) ** 2).mean(-1, keepdims=True)
    normed = (agg - mu) / np.sqrt(var + EPS) * gamma + beta
    res = np.einsum("bfji,oi->bfjo", f64, res_W)
    from scipy.special import erf  # noqa: PLC0415

    gelu = normed * 0.5 * (1.0 + erf(normed / np.sqrt(2.0)))
    return (gelu + res).astype(np.float32)


def kernel(f, W, pose_emb, gamma, beta, res_W, src, dst):
    f = np.asarray(f)
    W = np.asarray(W, np.float32)
    pose_emb = np.asarray(pose_emb, np.float32)
    gamma = np.asarray(gamma, np.float32)
    beta = np.asarray(beta, np.float32)
    res_W = np.asarray(res_W, np.float32)
    src = np.asarray(src)
    dst = np.asarray(dst)

    chain = np.array_equal(src, np.arange(1, J)) and np.array_equal(
        dst, np.arange(0, J - 1)
    )
    if not chain or f.shape != (B, FR, J, DIN):
        return _numpy_fallback(f, W, pose_emb, gamma, beta, res_W, src, dst)

    trivial_affine = bool(
        np.all(gamma == gamma.flat[0])
        and abs(gamma.flat[0] - 1.0) < 1e-12
        and np.all(beta == 0.0)
    )

    # Host prep: center W rows / pose so on-chip LN mean is exactly 0.
    Wc = W - W.mean(axis=2, keepdims=True)              # (E, Din, Dout)
    pc = pose_emb - pose_emb.mean(axis=1, keepdims=True)  # (J, Dout)
    w_host = np.ascontiguousarray(Wc.transpose(1, 0, 2).reshape(DIN, E * DOUT)).astype(
        ml_dtypes.bfloat16
    )
    # root joint (no incoming edge): gelu(LN(pose_52)*gamma+beta) is constant
    p52 = pc[J - 1].astype(np.float64)
    n52 = p52 / np.sqrt((p52 ** 2).mean() + EPS) * gamma.astype(np.float64) + beta
    from scipy.special import erf  # noqa: PLC0415

    g52 = (n52 * 0.5 * (1.0 + erf(n52 / np.sqrt(2.0)))).astype(np.float32)
    g52_host = g52.reshape(1, DOUT).astype(ml_dtypes.bfloat16)
    rw_host = np.ascontiguousarray(res_W.T).astype(ml_dtypes.bfloat16)  # (Din, Dout)
    pose_host = pc.reshape(1, J * DOUT).astype(ml_dtypes.bfloat16)

    # pre-transpose f: [core][tile*DIN, J*FRT] bf16 (DIN on partitions)
    f_bf = f.reshape(NCORES, NT, FRT, J, DIN).astype(ml_dtypes.bfloat16)
    f_host = np.ascontiguousarray(f_bf.transpose(0, 1, 4, 3, 2)).reshape(
        NCORES, NT * DIN, J * FRT
    )

    from concourse.bass_utils import run_bass_kernel_spmd  # noqa: PLC0415

    nc = _get_nc(trivial_affine)
    in_maps = []
    for c in range(NCORES):
        m = {
            "f": f_host[c],
            "w": w_host,
            "rw": rw_host,
            "pose": pose_host,
            "g52": g52_host,
        }
        if not trivial_affine:
            m["gam"] = gamma.reshape(1, DOUT)
            m["bet"] = beta.reshape(1, DOUT)
        in_maps.append(m)

    _CACHE["last_in_maps"] = in_maps
    res = run_bass_kernel_spmd(nc, in_maps, core_ids=list(range(NCORES)))
    outs = [
        res.results[c]["out"].astype(np.float32).reshape(FPC, J, DOUT)
        for c in range(NCORES)
    ]
    return np.concatenate(outs, axis=0).reshape(B, FR, J, DOUT)
